# revision 32
# baseline (speedup 1.0000x reference)
"""Multi-head self-attention (S=2048, B=2, D=1024, H=16) on 8 TRN2 NeuronCores.

Sharding: core c handles batch b = c//4 and head-quad g = c%4 (4 heads of 64).
Megatron-style: in_proj column-sliced, out_proj row-sliced; host sums the 8
partial outputs (bf16 partials) and adds the output bias. The V-projection
bias is folded into the host-side output bias (softmax rows sum to 1, so
P@(V + 1*bv) = P@V + 1*bv).

Per-core dataflow (matmul inputs bf16, accumulation fp32):
  - host supplies x^T (D-major) activations and pre-transposed weight slices
  - qpT/kpT computed head-major (m on partitions, seq on free)
  - vp computed seq-major with an interleaved ones column per head (65-wide
    blocks) so the PV matmul also produces softmax row-sums on partition 64
  - scores^T per (head-pair, 512-query-chunk, key-tile) in a packed psum tile
    (128, 2, 512); exp on ACT reads the pair in one op
  - normalization: DVE reciprocal of the row-sums then an SBUF->SBUF DMA
    partition-broadcast; DVE multiplies into attnT

Scheduling: the ACT exp stream (128 x 1147ns = 147us) and the PE stream
(~145us) are the rooflines. All input DMA goes on the single sync HWDGE
queue in strict priority order (the ~350GB/s per-core HBM pipe is shared,
so parallel queues only steal from the critical path). Minimal eager
projection starts attention at ~18us; all other projections and the
out-projection are paced background generators between attention
iterations. Tail copies route to the then-idle ACT engine.
"""

import math
from collections import deque
from contextlib import ExitStack

import numpy as np

S = 2048
B = 2
D = 1024
H = 16
DK = 64
HC = 4          # heads per core
M = HC * DK     # 256 head-dim columns per core
N_CORES = 8
KT = S // 128   # 16 key tiles
QQ = 4          # 512-wide query chunks

MM_DT = "bfloat16"   # dtype of matmul inputs

_compiled = None


def _build_program():
    import concourse.tile as tile
    from concourse import mybir, bacc

    f32 = mybir.dt.float32
    f32r = mybir.dt.float32r
    mdt = getattr(mybir.dt, MM_DT)
    EXP = mybir.ActivationFunctionType.Exp

    nc = bacc.Bacc("TRN2", target_bir_lowering=False, debug=False)

    xqT = nc.dram_tensor("xqT", [D, S], mdt, kind="ExternalInput").ap()
    xkT = nc.dram_tensor("xkT", [D, S], mdt, kind="ExternalInput").ap()
    xvT = nc.dram_tensor("xvT", [D, S], mdt, kind="ExternalInput").ap()
    wqT = nc.dram_tensor("wqT", [D, M], mdt, kind="ExternalInput").ap()
    wkT = nc.dram_tensor("wkT", [D, M], mdt, kind="ExternalInput").ap()
    wvT = nc.dram_tensor("wvT", [D, M], mdt, kind="ExternalInput").ap()
    bq = nc.dram_tensor("bq", [M], f32, kind="ExternalInput").ap()
    bk = nc.dram_tensor("bk", [M], f32, kind="ExternalInput").ap()
    woT = nc.dram_tensor("woT", [M, D], mdt, kind="ExternalInput").ap()
    out = nc.dram_tensor("out", [S, D], mdt, kind="ExternalOutput").ap()

    from concourse import library_config

    with tile.TileContext(nc) as tc, ExitStack() as ctx:
        const_pool = ctx.enter_context(tc.tile_pool(name="const", bufs=1))
        x_pool = ctx.enter_context(tc.tile_pool(name="x", bufs=16))
        e_pool = ctx.enter_context(tc.tile_pool(name="e", bufs=12))
        o_pool = ctx.enter_context(tc.tile_pool(name="o", bufs=3))
        r_pool = ctx.enter_context(tc.tile_pool(name="r", bufs=2))
        ps_a = ctx.enter_context(tc.tile_pool(name="ps_a", bufs=2, space="PSUM"))
        ps_b = ctx.enter_context(tc.tile_pool(name="ps_b", bufs=4, space="PSUM"))

        # ---- persistent SBUF tensors ----
        wq_sb = const_pool.tile([128, 8, M], mdt)
        wk_sb = const_pool.tile([128, 8, M], mdt)
        wv_sb = const_pool.tile([128, 8, M], mdt)
        wo_sb = const_pool.tile([128, 2, D], mdt)
        bq_sb = const_pool.tile([128, 2], f32)
        bk_sb = const_pool.tile([128, 2], f32)

        qpT = const_pool.tile([128, 2, S], mdt)   # [p, mt, s]
        kpT = const_pool.tile([128, 2, S], mdt)
        vp = const_pool.tile([128, KT, HC * 65], mdt)  # aug: 65-wide per head
        attnT = const_pool.tile([128, 2, S], mdt)

        # gpsimd ucode library with partition_broadcast (softmax epilogue)
        nc.gpsimd.load_library(library_config.attn)

        # ones columns of the augmented V (head h at column h*65+64)
        nc.vector.memset(
            vp[:, :, :].rearrange("p kt (h c) -> p kt h c", c=65)[:, :, :, 64:65], 1.0
        )

        # ---- input DMA in consumption order. The ~350GB/s per-core HBM pipe
        # is shared, so ordering IS prioritization. xq-h0 issues ride the
        # scalar HWDGE queue (idle until the first exp) so its 8 issues don't
        # serialize behind xk's on sync.
        nc.sync.dma_start(
            out=wk_sb[:, :, :], in_=wkT.rearrange("(kc p) m -> p kc m", p=128)
        )
        nc.sync.dma_start(
            out=wq_sb[:, :, :], in_=wqT.rearrange("(kc p) m -> p kc m", p=128)
        )
        def make_chunk(tag, i):
            return x_pool.tile([128, 1024], mdt, tag=tag, name=f"{tag}{i}")

        def load_cols(x_dr, xt, idx, ns, ne, eng):
            half, kc = divmod(idx, 8)
            fs = half * 1024
            eng.dma_start(
                out=xt[:, ns:ne],
                in_=x_dr[kc * 128:(kc + 1) * 128, fs + ns:fs + ne],
            )
            return xt

        # Single sync queue in consumption order (~350GB/s is shared, so
        # order IS priority). Only xq's first half is column-split — the
        # first scores need queries 0-511 + keys 0-1023; everything else
        # keeps whole [128,1024] chunks for 2KB DMA lines.
        xk_ch = [make_chunk("xk", i) for i in range(16)]
        xq_ch = [make_chunk("xq", i) for i in range(16)]
        xv_ch = [make_chunk("xv", i) for i in range(16)]
        for i in range(8):
            load_cols(xqT, xq_ch[i], i, 0, 512, nc.sync)
        for i in range(8):
            load_cols(xkT, xk_ch[i], i, 0, 1024, nc.sync)
        for i in range(8):
            load_cols(xqT, xq_ch[i], i, 512, 1024, nc.sync)
        # bias loads expand to many tiny descriptors (slow issue) — keep them
        # off the critical prefix on the (otherwise idle) scalar queue
        nc.scalar.dma_start(out=bk_sb[:, :], in_=bk.rearrange("(mt p) -> p mt", p=128))
        nc.scalar.dma_start(out=bq_sb[:, :], in_=bq.rearrange("(mt p) -> p mt", p=128))
        nc.scalar.dma_start(
            out=wv_sb[:, :, :], in_=wvT.rearrange("(kc p) m -> p kc m", p=128)
        )
        for i in range(8):
            load_cols(xvT, xv_ch[i], i, 0, 1024, nc.sync)
        for i in range(8, 16):
            load_cols(xkT, xk_ch[i], i, 0, 1024, nc.sync)
        for i in range(8, 16):
            load_cols(xvT, xv_ch[i], i, 0, 1024, nc.sync)
        for i in range(8, 16):
            load_cols(xqT, xq_ch[i], i, 0, 1024, nc.sync)
        nc.sync.dma_start(
            out=wo_sb[:, :, :], in_=woT.rearrange("(kc p) j -> p kc j", p=128)
        )

        # ---- projection chains (generators yield ~ns of PE work emitted) ----
        def gen_proj(w_sb, b_sb, p_sb, chunks, mt, half, nch, ns=None, ne=None):
            fs = half * 1024
            if ns is None:
                ns, ne = nch * 512, nch * 512 + 512
            width = ne - ns
            ps = ps_b.tile([128, 512], f32, tag="ps_small",
                           name=f"ps_p{mt}{half}{ns}")
            for kc in range(8):
                nc.tensor.matmul(
                    ps[:, 0:width],
                    w_sb[:, kc, mt * 128:(mt + 1) * 128],
                    chunks[half * 8 + kc][:, ns:ne],
                    start=(kc == 0),
                    stop=(kc == 7),
                )
                if kc < 7:
                    yield 220 * width // 512 + 90
            nc.vector.tensor_scalar_add(
                out=p_sb[:, mt, fs + ns:fs + ne],
                in0=ps[:, 0:width],
                scalar1=b_sb[:, mt:mt + 1],
            )
            yield 500

        # vp super-group: two key-tiles into one PSUM bank (the second chain's
        # first matmul uses start=False: its elements' has_written bits are
        # clear, so it overwrites rather than accumulates — one bank-clear for
        # the whole tile).
        def vp_group2(g):
            ps = ps_b.tile([128, 2, 256], f32, tag="ps_small", name="ps_v")
            for sub in range(2):
                kt = 2 * g + sub
                half, st = divmod(kt, 8)
                for kc in range(8):
                    nc.tensor.matmul(
                        ps[:, sub, 0:M],
                        xv_ch[half * 8 + kc][:, st * 128:(st + 1) * 128],
                        wv_sb[:, kc, :],
                        start=(sub == 0 and kc == 0),
                        stop=(sub == 1 and kc == 7),
                        skip_group_check=True,
                    )
                yield 900
            nc.vector.tensor_copy(
                out=vp[:, 2 * g:2 * g + 2, :]
                    .rearrange("p k (h c) -> p k h c", c=65)[:, :, :, 0:64],
                in_=ps[:, :, 0:M].rearrange("p k (h c) -> p k h c", c=64),
            )
            yield 100

        # ---- softmax epilogue, two phases ----
        # Phase A (at segment end): copy u out of PSUM so the banks free
        # immediately for the next segment's accumulators. Phase B (deferred
        # into the next segment): reciprocal of the row-sums, replicate across
        # 64 partitions on the otherwise idle GPSIMD engine, multiply.
        def flush_copies(u_tiles, use_act=False):
            rs = r_pool.tile([1, 2, 512], f32, tag="rs")
            us_t = []
            for hh in range(2):
                u = u_tiles[hh]
                nc.vector.tensor_copy(out=rs[:, hh, :], in_=u[64:65, :])
                us = r_pool.tile([64, 512], f32, tag="us")
                if use_act:
                    nc.scalar.copy(out=us[:, :], in_=u[0:64, :])
                else:
                    nc.vector.tensor_copy(out=us[:, :], in_=u[0:64, :])
                us_t.append(us)
            return rs, us_t

        def flush_norm(pair, qs, rs, us_t, on_pe=False):
            rr = r_pool.tile([1, 2, 512], f32, tag="rr")
            nc.vector.reciprocal_approx_fast(out=rr[:, :, :], in_=rs[:, :, :])
            rbs = r_pool.tile([64, 2, 512], f32, tag="rbs")
            nc.gpsimd.partition_broadcast(
                rbs[:, :, :], rr[0:1, :, :], channels=64
            )
            rbs_ap = [rbs[0:64, 0, :], rbs[0:64, 1, :]]
            for hh in range(2):
                with nc.allow_low_precision(reason="softmax normalize"):
                    nc.vector.tensor_tensor(
                        out=attnT[hh * 64:hh * 64 + 64, pair, qs:qs + 512],
                        in0=us_t[hh][0:64, :],
                        in1=rbs_ap[hh],
                        op=mybir.AluOpType.mult,
                    )

        # ---- out-projection (generator, per 128-row seq tile) ----
        def gen_outproj(sg, use_act=False):
            ot = o_pool.tile([128, D], mdt, tag="ot")
            for nch2 in range(2):
                po = ps_b.tile([128, 512], f32, tag="ps_small", name=f"po{sg}{nch2}")
                for kc in range(2):
                    nc.tensor.matmul(
                        po[:, :],
                        attnT[:, kc, sg * 128:(sg + 1) * 128],
                        wo_sb[:, kc, nch2 * 512:(nch2 + 1) * 512],
                        start=(kc == 0),
                        stop=(kc == 1),
                    )
                if use_act:
                    nc.scalar.copy(out=ot[:, nch2 * 512:(nch2 + 1) * 512], in_=po[:, :])
                else:
                    nc.vector.tensor_copy(
                        out=ot[:, nch2 * 512:(nch2 + 1) * 512], in_=po[:, :]
                    )
                yield 700
            nc.sync.dma_start(
                out=out[sg * 128:(sg + 1) * 128, :], in_=ot[:, :]
            )
            yield 100

        # ---- eager startup: just enough projection to start attention ----
        # kp keys 0-511, then qp queries 0-511
        for g in (
            gen_proj(wk_sb, bk_sb, kpT, xk_ch, 0, 0, 0),
            gen_proj(wq_sb, bq_sb, qpT, xq_ch, 0, 0, 0),
        ):
            for _ in g:
                pass

        # background work, ordered by deadline (pair0 carries everything that
        # pair1's first segment needs; the rest rides in pair1's slack)
        bg = deque([
            gen_proj(wk_sb, bk_sb, kpT, xk_ch, 0, 0, 1),  # keys 512-1023 (kt4)
            gen_proj(wk_sb, bk_sb, kpT, xk_ch, 0, 1, 0),  # keys 1024-1535 (qq0 kt8)
            gen_proj(wk_sb, bk_sb, kpT, xk_ch, 0, 1, 1),  # keys 1536-2047 (qq0 kt12)
            gen_proj(wq_sb, bq_sb, qpT, xq_ch, 0, 0, 1),  # q 512-1023 (qq1)
            gen_proj(wq_sb, bq_sb, qpT, xq_ch, 0, 1, 0),  # q 1024-1535 (qq2)
            gen_proj(wq_sb, bq_sb, qpT, xq_ch, 0, 1, 1),  # q 1536-2047 (qq3)
            gen_proj(wk_sb, bk_sb, kpT, xk_ch, 1, 0, 0),  # pair1 keys
            gen_proj(wk_sb, bk_sb, kpT, xk_ch, 1, 0, 1),
            gen_proj(wk_sb, bk_sb, kpT, xk_ch, 1, 1, 0),
            gen_proj(wk_sb, bk_sb, kpT, xk_ch, 1, 1, 1),
            gen_proj(wq_sb, bq_sb, qpT, xq_ch, 1, 0, 0),  # pair1-qq0 queries
        ])
        bg_p1 = [
            gen_proj(wq_sb, bq_sb, qpT, xq_ch, 1, 0, 1),  # pair1-qq1
            gen_proj(wq_sb, bq_sb, qpT, xq_ch, 1, 1, 0),  # pair1-qq2
            gen_proj(wq_sb, bq_sb, qpT, xq_ch, 1, 1, 1),  # pair1-qq3
        ]

        def pump(budget):
            while budget > 0 and bg:
                try:
                    budget -= next(bg[0])
                except StopIteration:
                    bg.popleft()

        def scores_exp(pair, qs, kt):
            sc = ps_a.tile([128, 2, 512], f32, tag="ps_main")
            ks = kt * 128
            for hh in range(2):
                po = hh * 64
                nc.tensor.matmul(
                    sc[:, hh, :],
                    kpT[po:po + 64, pair, ks:ks + 128],
                    qpT[po:po + 64, pair, qs:qs + 512],
                    start=True,
                    stop=True,
                )
            et = e_pool.tile([128, 2, 512], mdt, tag="et")
            nc.scalar.activation(out=et[:, :, :], in_=sc[:, :, :], func=EXP)
            return et

        def pv(pair, u_tiles, kt, et):
            for hh in range(2):
                h = 2 * pair + hh
                nc.tensor.matmul(
                    u_tiles[hh][0:65, :],
                    vp[:, kt, h * 65:(h + 1) * 65],
                    et[:, hh, :],
                    start=(kt == 0),
                    stop=(kt == KT - 1),
                )

        def new_u(pair, qq):
            return [
                ps_b.tile([65, 512], f32, tag="ps_small", name=f"u_{pair}{qq}{hh}")
                for hh in range(2)
            ]

        # ---- first segment (pair0, qq0): vp is built here just-in-time,
        # one key-tile sub-chain per iteration. PV lags the exp stream by
        # PV_LAG iterations so DMA-gated vp work never sits ahead of ready
        # work in the in-order PE queue.
        PV_LAG = 7
        vq = deque(vp_group2(g) for g in range(8))

        def vpump(budget):
            while budget > 0 and vq:
                try:
                    budget -= next(vq[0])
                except StopIteration:
                    vq.popleft()

        u0 = new_u(0, 0)
        ets = {}
        for j in range(KT):
            ets[j] = scores_exp(0, 0, j)
            if j >= 5:
                vpump(950)
            pump(700 if j <= 8 else 500)
            if j >= PV_LAG:
                pv(0, u0, j - PV_LAG, ets.pop(j - PV_LAG))
        vpump(1 << 30)
        for j in range(KT - PV_LAG, KT):
            pv(0, u0, j, ets.pop(j))
        pend_norm = (0, 0) + flush_copies(u0)

        # ---- remaining segments ----
        for pair in range(2):
            for qq in range(QQ):
                if pair == 0 and qq == 0:
                    continue
                qs = qq * 512
                last_seg = pair == 1 and qq == QQ - 1
                u_tiles = new_u(pair, qq)
                for kt in range(KT):
                    et = scores_exp(pair, qs, kt)
                    if kt == 1 and pend_norm is not None:
                        flush_norm(*pend_norm)
                        pend_norm = None
                    if kt >= 2:
                        pump(650 if pair == 1 else 450)
                    pv(pair, u_tiles, kt, et)
                pend_norm = (pair, qs) + flush_copies(
                    u_tiles, use_act=last_seg
                )
                if pair == 0 and qq == QQ - 1:
                    bg.extend(bg_p1)
                if pair == 1:
                    for sg in range(qq * 4, qq * 4 + 4):
                        bg.append(gen_outproj(sg, use_act=last_seg))
        # tail: last segment's normalize + remaining out-projection
        flush_norm(*pend_norm, on_pe=True)
        while bg:
            pump(1 << 30)

    nc.compile()
    return nc


def _get_compiled():
    global _compiled
    if _compiled is None:
        _compiled = _build_program()
    return _compiled


def _make_in_maps(q, k, v, in_proj_w, in_proj_b, out_proj_w):
    import ml_dtypes

    mdt_np = np.dtype(ml_dtypes.bfloat16) if MM_DT == "bfloat16" else np.float32

    def cvt(a):
        return np.ascontiguousarray(a).astype(mdt_np)

    xT = {}
    for b in range(B):
        xT[b] = (
            cvt(q[:, b, :].T),
            cvt(k[:, b, :].T),
            cvt(v[:, b, :].T),
        )
    scale = 1.0 / math.sqrt(DK)
    in_maps = []
    for c in range(N_CORES):
        b, g = divmod(c, HC)
        cols = slice(g * M, (g + 1) * M)
        in_maps.append({
            "xqT": xT[b][0],
            "xkT": xT[b][1],
            "xvT": xT[b][2],
            "wqT": cvt((in_proj_w[0 * D:1 * D][cols] * scale).T),
            "wkT": cvt(in_proj_w[1 * D:2 * D][cols].T),
            "wvT": cvt(in_proj_w[2 * D:3 * D][cols].T),
            "bq": np.ascontiguousarray(in_proj_b[0 * D:1 * D][cols] * scale),
            "bk": np.ascontiguousarray(in_proj_b[1 * D:2 * D][cols]),
            "woT": cvt(out_proj_w[:, g * M:(g + 1) * M].T),
        })
    return in_maps


def kernel(q, k, v, in_proj_w, in_proj_b, out_proj_w, out_proj_b):
    from concourse.bass_utils import run_bass_kernel_spmd

    q = np.asarray(q, dtype=np.float32)
    k = np.asarray(k, dtype=np.float32)
    v = np.asarray(v, dtype=np.float32)
    in_proj_w = np.asarray(in_proj_w, dtype=np.float32)
    in_proj_b = np.asarray(in_proj_b, dtype=np.float32)
    out_proj_w = np.asarray(out_proj_w, dtype=np.float32)
    out_proj_b = np.asarray(out_proj_b, dtype=np.float32)

    nc = _get_compiled()
    in_maps = _make_in_maps(q, k, v, in_proj_w, in_proj_b, out_proj_w)

    res = run_bass_kernel_spmd(nc, in_maps, core_ids=list(range(N_CORES)))

    # V-projection bias folded here: softmax rows sum to 1, so the bv term
    # contributes out_proj_w @ bv to every output row.
    bias = out_proj_b + out_proj_w @ in_proj_b[2 * D:3 * D]
    out = np.broadcast_to(bias.astype(np.float32), (S, B, D)).copy()
    for c in range(N_CORES):
        out[:, c // HC, :] += res.results[c]["out"].astype(np.float32)
    return out


# revision 33
# speedup vs baseline: 1.0111x; 1.0111x over previous
"""Multi-head self-attention (S=2048, B=2, D=1024, H=16) on 8 TRN2 NeuronCores.

Sharding: core c handles batch b = c//4 and head-quad g = c%4 (4 heads of 64).
Megatron-style: in_proj column-sliced, out_proj row-sliced; host sums the 8
partial outputs (bf16 partials) and adds the output bias. The V-projection
bias is folded into the host-side output bias (softmax rows sum to 1, so
P@(V + 1*bv) = P@V + 1*bv).

Per-core dataflow (matmul inputs bf16, accumulation fp32):
  - host supplies x^T (D-major) activations and pre-transposed weight slices
  - qpT/kpT computed head-major (m on partitions, seq on free)
  - vp computed seq-major with an interleaved ones column per head (65-wide
    blocks) so the PV matmul also produces softmax row-sums on partition 64
  - scores^T per (head-pair, 512-query-chunk, key-tile) in a packed psum tile
    (128, 2, 512); exp on ACT reads the pair in one op
  - normalization: DVE reciprocal of the row-sums then an SBUF->SBUF DMA
    partition-broadcast; DVE multiplies into attnT

Scheduling: the ACT exp stream (128 x 1147ns = 147us) and the PE stream
(~145us) are the rooflines. All input DMA goes on the single sync HWDGE
queue in strict priority order (the ~350GB/s per-core HBM pipe is shared,
so parallel queues only steal from the critical path). Minimal eager
projection starts attention at ~18us; all other projections and the
out-projection are paced background generators between attention
iterations. Tail copies route to the then-idle ACT engine.
"""

import math
from collections import deque
from contextlib import ExitStack

import numpy as np

S = 2048
B = 2
D = 1024
H = 16
DK = 64
HC = 4          # heads per core
M = HC * DK     # 256 head-dim columns per core
N_CORES = 8
KT = S // 128   # 16 key tiles
QQ = 4          # 512-wide query chunks

MM_DT = "bfloat16"   # dtype of matmul inputs

_compiled = None


def _build_program():
    import concourse.tile as tile
    from concourse import mybir, bacc

    f32 = mybir.dt.float32
    f32r = mybir.dt.float32r
    mdt = getattr(mybir.dt, MM_DT)
    EXP = mybir.ActivationFunctionType.Exp

    nc = bacc.Bacc("TRN2", target_bir_lowering=False, debug=False)

    xqT = nc.dram_tensor("xqT", [D, S], mdt, kind="ExternalInput").ap()
    xkT = nc.dram_tensor("xkT", [D, S], mdt, kind="ExternalInput").ap()
    xvT = nc.dram_tensor("xvT", [D, S], mdt, kind="ExternalInput").ap()
    wqT = nc.dram_tensor("wqT", [D, M], mdt, kind="ExternalInput").ap()
    wkT = nc.dram_tensor("wkT", [D, M], mdt, kind="ExternalInput").ap()
    wvT = nc.dram_tensor("wvT", [D, M], mdt, kind="ExternalInput").ap()
    bq = nc.dram_tensor("bq", [M], f32, kind="ExternalInput").ap()
    bk = nc.dram_tensor("bk", [M], f32, kind="ExternalInput").ap()
    woT = nc.dram_tensor("woT", [M, D], mdt, kind="ExternalInput").ap()
    out = nc.dram_tensor("out", [S, D], mdt, kind="ExternalOutput").ap()

    from concourse import library_config

    with tile.TileContext(nc) as tc, ExitStack() as ctx:
        const_pool = ctx.enter_context(tc.tile_pool(name="const", bufs=1))
        x_pool = ctx.enter_context(tc.tile_pool(name="x", bufs=16))
        e_pool = ctx.enter_context(tc.tile_pool(name="e", bufs=12))
        o_pool = ctx.enter_context(tc.tile_pool(name="o", bufs=3))
        r_pool = ctx.enter_context(tc.tile_pool(name="r", bufs=2))
        ps_a = ctx.enter_context(tc.tile_pool(name="ps_a", bufs=2, space="PSUM"))
        ps_b = ctx.enter_context(tc.tile_pool(name="ps_b", bufs=4, space="PSUM"))

        # ---- persistent SBUF tensors ----
        wq_sb = const_pool.tile([128, 8, M], mdt)
        wk_sb = const_pool.tile([128, 8, M], mdt)
        wv_sb = const_pool.tile([128, 8, M], mdt)
        wo_sb = const_pool.tile([128, 2, D], mdt)
        bq_sb = const_pool.tile([128, 2], f32)
        bk_sb = const_pool.tile([128, 2], f32)

        qpT = const_pool.tile([128, 2, S], mdt)   # [p, mt, s]
        kpT = const_pool.tile([128, 2, S], mdt)
        vp = const_pool.tile([128, KT, HC * 65], mdt)  # aug: 65-wide per head
        attnT = const_pool.tile([128, 2, S], mdt)

        # gpsimd ucode library with partition_broadcast (softmax epilogue)
        nc.gpsimd.load_library(library_config.attn)

        # ones columns of the augmented V (head h at column h*65+64)
        nc.vector.memset(
            vp[:, :, :].rearrange("p kt (h c) -> p kt h c", c=65)[:, :, :, 64:65], 1.0
        )

        # ---- input DMA in consumption order. The ~350GB/s per-core HBM pipe
        # is shared, so ordering IS prioritization. xq-h0 issues ride the
        # scalar HWDGE queue (idle until the first exp) so its 8 issues don't
        # serialize behind xk's on sync.
        nc.sync.dma_start(
            out=wk_sb[:, :, :], in_=wkT.rearrange("(kc p) m -> p kc m", p=128)
        )
        nc.sync.dma_start(
            out=wq_sb[:, :, :], in_=wqT.rearrange("(kc p) m -> p kc m", p=128)
        )
        def make_chunk(tag, i):
            return x_pool.tile([128, 1024], mdt, tag=tag, name=f"{tag}{i}")

        def load_cols(x_dr, xt, idx, ns, ne, eng):
            half, kc = divmod(idx, 8)
            fs = half * 1024
            eng.dma_start(
                out=xt[:, ns:ne],
                in_=x_dr[kc * 128:(kc + 1) * 128, fs + ns:fs + ne],
            )
            return xt

        # Whole [128,1024] chunks keep 2KB DMA lines (column slices halve
        # line size and throttle the stream); xq-h0 rides the scalar HWDGE
        # queue in parallel, everything else in deadline order on sync (the
        # ~350GB/s per-core HBM pipe is shared, so order IS priority).
        xk_ch = [make_chunk("xk", i) for i in range(16)]
        xq_ch = [make_chunk("xq", i) for i in range(16)]
        xv_ch = [make_chunk("xv", i) for i in range(16)]
        for i in range(8):
            load_cols(xkT, xk_ch[i], i, 0, 1024, nc.sync)
        for i in range(8):
            load_cols(xqT, xq_ch[i], i, 0, 1024, nc.scalar)
        # bias loads expand to many tiny descriptors (slow issue) — keep them
        # behind xq on the scalar queue
        nc.scalar.dma_start(out=bk_sb[:, :], in_=bk.rearrange("(mt p) -> p mt", p=128))
        nc.scalar.dma_start(out=bq_sb[:, :], in_=bq.rearrange("(mt p) -> p mt", p=128))
        nc.scalar.dma_start(
            out=wv_sb[:, :, :], in_=wvT.rearrange("(kc p) m -> p kc m", p=128)
        )
        for i in range(8):
            load_cols(xvT, xv_ch[i], i, 0, 1024, nc.sync)
        for i in range(8, 16):
            load_cols(xkT, xk_ch[i], i, 0, 1024, nc.sync)
        for i in range(8, 16):
            load_cols(xvT, xv_ch[i], i, 0, 1024, nc.sync)
        for i in range(8, 16):
            load_cols(xqT, xq_ch[i], i, 0, 1024, nc.sync)
        nc.sync.dma_start(
            out=wo_sb[:, :, :], in_=woT.rearrange("(kc p) j -> p kc j", p=128)
        )

        # ---- projection chains (generators yield ~ns of PE work emitted) ----
        def gen_proj(w_sb, b_sb, p_sb, chunks, mt, half, nch, ns=None, ne=None):
            fs = half * 1024
            if ns is None:
                ns, ne = nch * 512, nch * 512 + 512
            width = ne - ns
            ps = ps_b.tile([128, 512], f32, tag="ps_small",
                           name=f"ps_p{mt}{half}{ns}")
            for kc in range(8):
                nc.tensor.matmul(
                    ps[:, 0:width],
                    w_sb[:, kc, mt * 128:(mt + 1) * 128],
                    chunks[half * 8 + kc][:, ns:ne],
                    start=(kc == 0),
                    stop=(kc == 7),
                )
                if kc < 7:
                    yield 220 * width // 512 + 90
            nc.vector.tensor_scalar_add(
                out=p_sb[:, mt, fs + ns:fs + ne],
                in0=ps[:, 0:width],
                scalar1=b_sb[:, mt:mt + 1],
            )
            yield 500

        # vp super-group: two key-tiles into one PSUM bank (the second chain's
        # first matmul uses start=False: its elements' has_written bits are
        # clear, so it overwrites rather than accumulates — one bank-clear for
        # the whole tile).
        def vp_group2(g):
            ps = ps_b.tile([128, 2, 256], f32, tag="ps_small", name="ps_v")
            for sub in range(2):
                kt = 2 * g + sub
                half, st = divmod(kt, 8)
                for kc in range(8):
                    nc.tensor.matmul(
                        ps[:, sub, 0:M],
                        xv_ch[half * 8 + kc][:, st * 128:(st + 1) * 128],
                        wv_sb[:, kc, :],
                        start=(sub == 0 and kc == 0),
                        stop=(sub == 1 and kc == 7),
                        skip_group_check=True,
                    )
                yield 900
            nc.vector.tensor_copy(
                out=vp[:, 2 * g:2 * g + 2, :]
                    .rearrange("p k (h c) -> p k h c", c=65)[:, :, :, 0:64],
                in_=ps[:, :, 0:M].rearrange("p k (h c) -> p k h c", c=64),
            )
            yield 100

        # ---- softmax epilogue, two phases ----
        # Phase A (at segment end): copy u out of PSUM so the banks free
        # immediately for the next segment's accumulators. Phase B (deferred
        # into the next segment): reciprocal of the row-sums, replicate across
        # 64 partitions on the otherwise idle GPSIMD engine, multiply.
        def flush_copies(u_tiles, use_act=False):
            rs = r_pool.tile([1, 2, 512], f32, tag="rs")
            us_t = []
            for hh in range(2):
                u = u_tiles[hh]
                nc.vector.tensor_copy(out=rs[:, hh, :], in_=u[64:65, :])
                us = r_pool.tile([64, 512], f32, tag="us")
                if use_act:
                    nc.scalar.copy(out=us[:, :], in_=u[0:64, :])
                else:
                    nc.vector.tensor_copy(out=us[:, :], in_=u[0:64, :])
                us_t.append(us)
            return rs, us_t

        def flush_norm(pair, qs, rs, us_t, on_pe=False):
            rr = r_pool.tile([1, 2, 512], f32, tag="rr")
            nc.vector.reciprocal_approx_fast(out=rr[:, :, :], in_=rs[:, :, :])
            rbs = r_pool.tile([64, 2, 512], f32, tag="rbs")
            nc.gpsimd.partition_broadcast(
                rbs[:, :, :], rr[0:1, :, :], channels=64
            )
            rbs_ap = [rbs[0:64, 0, :], rbs[0:64, 1, :]]
            for hh in range(2):
                with nc.allow_low_precision(reason="softmax normalize"):
                    nc.vector.tensor_tensor(
                        out=attnT[hh * 64:hh * 64 + 64, pair, qs:qs + 512],
                        in0=us_t[hh][0:64, :],
                        in1=rbs_ap[hh],
                        op=mybir.AluOpType.mult,
                    )

        # ---- out-projection (generator, per 128-row seq tile) ----
        def gen_outproj(sg, use_act=False):
            ot = o_pool.tile([128, D], mdt, tag="ot")
            for nch2 in range(2):
                po = ps_b.tile([128, 512], f32, tag="ps_small", name=f"po{sg}{nch2}")
                for kc in range(2):
                    nc.tensor.matmul(
                        po[:, :],
                        attnT[:, kc, sg * 128:(sg + 1) * 128],
                        wo_sb[:, kc, nch2 * 512:(nch2 + 1) * 512],
                        start=(kc == 0),
                        stop=(kc == 1),
                    )
                if use_act:
                    nc.scalar.copy(out=ot[:, nch2 * 512:(nch2 + 1) * 512], in_=po[:, :])
                else:
                    nc.vector.tensor_copy(
                        out=ot[:, nch2 * 512:(nch2 + 1) * 512], in_=po[:, :]
                    )
                yield 700
            nc.sync.dma_start(
                out=out[sg * 128:(sg + 1) * 128, :], in_=ot[:, :]
            )
            yield 100

        # ---- eager startup: just enough projection to start attention ----
        # kp keys 0-511, then qp queries 0-511
        for g in (
            gen_proj(wk_sb, bk_sb, kpT, xk_ch, 0, 0, 0),
            gen_proj(wq_sb, bq_sb, qpT, xq_ch, 0, 0, 0),
        ):
            for _ in g:
                pass

        # background work, ordered by deadline (pair0 carries everything that
        # pair1's first segment needs; the rest rides in pair1's slack)
        bg = deque([
            gen_proj(wk_sb, bk_sb, kpT, xk_ch, 0, 0, 1),  # keys 512-1023 (kt4)
            gen_proj(wk_sb, bk_sb, kpT, xk_ch, 0, 1, 0),  # keys 1024-1535 (qq0 kt8)
            gen_proj(wk_sb, bk_sb, kpT, xk_ch, 0, 1, 1),  # keys 1536-2047 (qq0 kt12)
            gen_proj(wq_sb, bq_sb, qpT, xq_ch, 0, 0, 1),  # q 512-1023 (qq1)
            gen_proj(wq_sb, bq_sb, qpT, xq_ch, 0, 1, 0),  # q 1024-1535 (qq2)
            gen_proj(wq_sb, bq_sb, qpT, xq_ch, 0, 1, 1),  # q 1536-2047 (qq3)
            gen_proj(wk_sb, bk_sb, kpT, xk_ch, 1, 0, 0),  # pair1 keys
            gen_proj(wk_sb, bk_sb, kpT, xk_ch, 1, 0, 1),
            gen_proj(wk_sb, bk_sb, kpT, xk_ch, 1, 1, 0),
            gen_proj(wk_sb, bk_sb, kpT, xk_ch, 1, 1, 1),
            gen_proj(wq_sb, bq_sb, qpT, xq_ch, 1, 0, 0),  # pair1-qq0 queries
        ])
        bg_p1 = [
            gen_proj(wq_sb, bq_sb, qpT, xq_ch, 1, 0, 1),  # pair1-qq1
            gen_proj(wq_sb, bq_sb, qpT, xq_ch, 1, 1, 0),  # pair1-qq2
            gen_proj(wq_sb, bq_sb, qpT, xq_ch, 1, 1, 1),  # pair1-qq3
        ]

        def pump(budget):
            while budget > 0 and bg:
                try:
                    budget -= next(bg[0])
                except StopIteration:
                    bg.popleft()

        def scores_exp(pair, qs, kt):
            sc = ps_a.tile([128, 2, 512], f32, tag="ps_main")
            ks = kt * 128
            for hh in range(2):
                po = hh * 64
                nc.tensor.matmul(
                    sc[:, hh, :],
                    kpT[po:po + 64, pair, ks:ks + 128],
                    qpT[po:po + 64, pair, qs:qs + 512],
                    start=True,
                    stop=True,
                )
            et = e_pool.tile([128, 2, 512], mdt, tag="et")
            nc.scalar.activation(out=et[:, :, :], in_=sc[:, :, :], func=EXP)
            return et

        def pv(pair, u_tiles, kt, et):
            for hh in range(2):
                h = 2 * pair + hh
                nc.tensor.matmul(
                    u_tiles[hh][0:65, :],
                    vp[:, kt, h * 65:(h + 1) * 65],
                    et[:, hh, :],
                    start=(kt == 0),
                    stop=(kt == KT - 1),
                )

        def new_u(pair, qq):
            return [
                ps_b.tile([65, 512], f32, tag="ps_small", name=f"u_{pair}{qq}{hh}")
                for hh in range(2)
            ]

        # ---- first segment (pair0, qq0): vp is built here just-in-time,
        # one key-tile sub-chain per iteration. PV lags the exp stream by
        # PV_LAG iterations so DMA-gated vp work never sits ahead of ready
        # work in the in-order PE queue.
        PV_LAG = 7
        vq = deque(vp_group2(g) for g in range(8))

        def vpump(budget):
            while budget > 0 and vq:
                try:
                    budget -= next(vq[0])
                except StopIteration:
                    vq.popleft()

        u0 = new_u(0, 0)
        ets = {}
        for j in range(KT):
            ets[j] = scores_exp(0, 0, j)
            if j >= 5:
                vpump(950)
            pump(700 if j <= 8 else 500)
            if j >= PV_LAG:
                pv(0, u0, j - PV_LAG, ets.pop(j - PV_LAG))
        vpump(1 << 30)
        for j in range(KT - PV_LAG, KT):
            pv(0, u0, j, ets.pop(j))
        pend_norm = (0, 0) + flush_copies(u0)

        # ---- remaining segments ----
        for pair in range(2):
            for qq in range(QQ):
                if pair == 0 and qq == 0:
                    continue
                qs = qq * 512
                last_seg = pair == 1 and qq == QQ - 1
                u_tiles = new_u(pair, qq)
                for kt in range(KT):
                    et = scores_exp(pair, qs, kt)
                    if kt == 1 and pend_norm is not None:
                        flush_norm(*pend_norm)
                        pend_norm = None
                    if kt >= 2:
                        pump(650 if pair == 1 else 450)
                    pv(pair, u_tiles, kt, et)
                pend_norm = (pair, qs) + flush_copies(
                    u_tiles, use_act=last_seg
                )
                if pair == 0 and qq == QQ - 1:
                    bg.extend(bg_p1)
                if pair == 1:
                    for sg in range(qq * 4, qq * 4 + 4):
                        bg.append(gen_outproj(sg, use_act=last_seg))
        # tail: last segment's normalize + remaining out-projection
        flush_norm(*pend_norm, on_pe=True)
        while bg:
            pump(1 << 30)

    nc.compile()
    return nc


def _get_compiled():
    global _compiled
    if _compiled is None:
        _compiled = _build_program()
    return _compiled


def _make_in_maps(q, k, v, in_proj_w, in_proj_b, out_proj_w):
    import ml_dtypes

    mdt_np = np.dtype(ml_dtypes.bfloat16) if MM_DT == "bfloat16" else np.float32

    def cvt(a):
        return np.ascontiguousarray(a).astype(mdt_np)

    xT = {}
    for b in range(B):
        xT[b] = (
            cvt(q[:, b, :].T),
            cvt(k[:, b, :].T),
            cvt(v[:, b, :].T),
        )
    scale = 1.0 / math.sqrt(DK)
    in_maps = []
    for c in range(N_CORES):
        b, g = divmod(c, HC)
        cols = slice(g * M, (g + 1) * M)
        in_maps.append({
            "xqT": xT[b][0],
            "xkT": xT[b][1],
            "xvT": xT[b][2],
            "wqT": cvt((in_proj_w[0 * D:1 * D][cols] * scale).T),
            "wkT": cvt(in_proj_w[1 * D:2 * D][cols].T),
            "wvT": cvt(in_proj_w[2 * D:3 * D][cols].T),
            "bq": np.ascontiguousarray(in_proj_b[0 * D:1 * D][cols] * scale),
            "bk": np.ascontiguousarray(in_proj_b[1 * D:2 * D][cols]),
            "woT": cvt(out_proj_w[:, g * M:(g + 1) * M].T),
        })
    return in_maps


def kernel(q, k, v, in_proj_w, in_proj_b, out_proj_w, out_proj_b):
    from concourse.bass_utils import run_bass_kernel_spmd

    q = np.asarray(q, dtype=np.float32)
    k = np.asarray(k, dtype=np.float32)
    v = np.asarray(v, dtype=np.float32)
    in_proj_w = np.asarray(in_proj_w, dtype=np.float32)
    in_proj_b = np.asarray(in_proj_b, dtype=np.float32)
    out_proj_w = np.asarray(out_proj_w, dtype=np.float32)
    out_proj_b = np.asarray(out_proj_b, dtype=np.float32)

    nc = _get_compiled()
    in_maps = _make_in_maps(q, k, v, in_proj_w, in_proj_b, out_proj_w)

    res = run_bass_kernel_spmd(nc, in_maps, core_ids=list(range(N_CORES)))

    # V-projection bias folded here: softmax rows sum to 1, so the bv term
    # contributes out_proj_w @ bv to every output row.
    bias = out_proj_b + out_proj_w @ in_proj_b[2 * D:3 * D]
    out = np.broadcast_to(bias.astype(np.float32), (S, B, D)).copy()
    for c in range(N_CORES):
        out[:, c // HC, :] += res.results[c]["out"].astype(np.float32)
    return out
